# revision 8
# baseline (speedup 1.0000x reference)
"""Trainium2 Bass kernel for batched attention.

Problem: b=16 batches of softmax(Q K^T / sqrt(128)) V with n=m=2048, d=dv=128,
fp32 inputs/outputs.

Sharding: batch dim across 8 NeuronCores (2 batches per core), no comms.

Per-core algorithm (per batch):
  1. Load Q, K naturally ([n,128] tiles), transpose via PE (identity matmul)
     to get Q^T, K^T in SBUF with d on partitions.
  2. MM1: S^T[mtile, n] = (K^T chunk).T-stationary x Q^T-moving, in float32r
     (one-pass FP22 matmul; ~1e-4 relative accuracy, 4x faster than true fp32).
  3. exp on ScalarE, fused temperature scale, PSUM->SBUF, output fp16 P^T.
  4. MM2: O[ntile, 129] accumulated over m chunks with stationary P^T chunk and
     moving [V | ones] fp16; column 128 gives the softmax denominator for free.
  5. DVE reciprocal + per-partition scale, store O naturally.
"""

import numpy as np

B = 16
N_CORES = 8
B_LOC = B // N_CORES  # 2 batches per core
N = 2048  # queries per batch
M = 2048  # keys per batch
D = 128   # head dim
NT = N // 128  # 16 n-tiles
MT = M // 128  # 16 m-tiles
INV_TEMP = 1.0 / 11.313708498984761  # 1/sqrt(128)

_CACHE = {}


def _build():
    import concourse.bacc as bacc
    import concourse.mybir as mybir
    import concourse.tile as tile
    from concourse.masks import make_identity

    f32 = mybir.dt.float32
    f32r = mybir.dt.float32r
    f16 = mybir.dt.float16

    nc = bacc.Bacc("TRN2", target_bir_lowering=False, debug=False,
                   num_devices=N_CORES)
    q_dram = nc.dram_tensor("queries", [B_LOC, N, D], f32, kind="ExternalInput")
    k_dram = nc.dram_tensor("keys", [B_LOC, M, D], f32, kind="ExternalInput")
    v_dram = nc.dram_tensor("values", [B_LOC, M, D], f32, kind="ExternalInput")
    o_dram = nc.dram_tensor("out", [B_LOC, N, D], f32, kind="ExternalOutput")

    with tile.TileContext(nc) as tc:
        with (
            tc.tile_pool(name="const", bufs=1) as const_pool,
            tc.tile_pool(name="nat", bufs=4) as nat_pool,
            tc.tile_pool(name="qT", bufs=2) as qT_pool,
            tc.tile_pool(name="kT", bufs=2) as kT_pool,
            tc.tile_pool(name="vo", bufs=2) as vo_pool,
            tc.tile_pool(name="pT", bufs=18) as pT_pool,
            tc.tile_pool(name="oall", bufs=2) as o_pool,
            tc.tile_pool(name="small", bufs=8) as small_pool,
            tc.tile_pool(name="psS", bufs=2, space="PSUM") as psS_pool,
            tc.tile_pool(name="psO", bufs=2, space="PSUM") as psO_pool,
        ):
            psT_pool = psO_pool  # transposes bounce through the psO slots
            ident = const_pool.tile([128, 128], f32)
            make_identity(nc, ident[:])

            for b in range(B_LOC):
                # ---- load Q, K naturally: SBUF [128, c*128] with free=(chunk, d)
                q_nat = nat_pool.tile([128, NT * 128], f32, tag="nat")
                k_nat = nat_pool.tile([128, MT * 128], f32, tag="nat")
                nc.sync.dma_start(
                    q_nat[:].rearrange("p (c d) -> p c d", d=128),
                    q_dram[b].rearrange("(c p) d -> p c d", p=128))
                nc.sync.dma_start(
                    k_nat[:].rearrange("p (c d) -> p c d", d=128),
                    k_dram[b].rearrange("(c p) d -> p c d", p=128))

                # ---- load V with cast to fp16, interleaved with a ones column
                vo = vo_pool.tile([128, MT * 129], f16)
                nc.gpsimd.dma_start(
                    vo[:].rearrange("p (c w) -> p c w", w=129)[:, :, 0:128],
                    v_dram[b].rearrange("(c p) d -> p c d", p=128))
                nc.vector.memset(
                    vo[:].rearrange("p (c w) -> p c w", w=129)[:, :, 128:129], 1.0)

                # ---- transpose Q, K via PE into [d, seq] layout
                # float32r: the DVE copy rounds to FP22 so MM1 can run the
                # one-pass reduced-precision fp32 matmul (walrus requires the
                # producer to do the rounding).
                qT = qT_pool.tile([128, N], f32r)
                kT = kT_pool.tile([128, M], f32r)
                for c in range(NT):
                    pst = psT_pool.tile([128, 1024], f32, tag="psO")
                    nc.tensor.transpose(pst[:, 0:128],
                                        q_nat[:, c * 128:(c + 1) * 128],
                                        ident[:])
                    nc.vector.tensor_copy(qT[:, c * 128:(c + 1) * 128],
                                          pst[:, 0:128])
                for c in range(MT):
                    pst = psT_pool.tile([128, 1024], f32, tag="psO")
                    nc.tensor.transpose(pst[:, 0:128],
                                        k_nat[:, c * 128:(c + 1) * 128],
                                        ident[:])
                    nc.vector.tensor_copy(kT[:, c * 128:(c + 1) * 128],
                                          pst[:, 0:128])

                # ---- MM1 (S^T chunks, float32r) + exp -> P^T fp16
                pTs = []
                for c in range(MT):
                    pT = pT_pool.tile([128, N], f16, tag="pT")
                    pTs.append(pT)
                    for h in range(2):  # halves of n: [128, 1024] PSUM tiles
                        psS = psS_pool.tile([128, 1024], f32)
                        for j in range(2):
                            nc.tensor.matmul(
                                psS[:, j * 512:(j + 1) * 512],
                                kT[:, c * 128:(c + 1) * 128],
                                qT[:, h * 1024 + j * 512:
                                   h * 1024 + (j + 1) * 512],
                                start=True, stop=True)
                        nc.scalar.activation(
                            pT[:, h * 1024:(h + 1) * 1024], psS[:],
                            mybir.ActivationFunctionType.Exp, scale=INV_TEMP)

                # ---- MM2: O tiles + denominator via ones column.
                # Row-split into two K=64 matmuls on disjoint PE row groups so
                # the LDWEIGHTS of one half overlaps the other half's stream
                # (PE pulls LDWEIGHTS ahead when row groups differ). Each half
                # accumulates into its own PSUM bank; DVE merges at drain.
                o_all = o_pool.tile([128, NT * 128], f32)
                for t in range(NT):
                    psO = psO_pool.tile([128, 1024], f32, tag="psO")
                    a = psO[:, 0:129]       # bank 0
                    b2 = psO[:, 512:641]    # bank 1
                    for c in range(MT):
                        nc.tensor.matmul(
                            a,
                            pTs[c][0:64, t * 128:(t + 1) * 128],
                            vo[0:64, c * 129:(c + 1) * 129],
                            start=(c == 0), stop=(c == MT - 1),
                            tile_position=(0, 0))
                        nc.tensor.matmul(
                            b2,
                            pTs[c][64:128, t * 128:(t + 1) * 128],
                            vo[64:128, c * 129:(c + 1) * 129],
                            start=(c == 0), stop=(c == MT - 1),
                            tile_position=(64, 0))
                    ob = small_pool.tile([128, 129], f32, tag="ob")
                    nc.vector.tensor_copy(ob[:], b2)
                    osum = small_pool.tile([128, 129], f32, tag="osum")
                    nc.vector.tensor_add(osum[:], a, ob[:])
                    recip = small_pool.tile([128, 1], f32, tag="recip")
                    nc.vector.reciprocal(recip[:], osum[:, 128:129])
                    nc.vector.tensor_scalar_mul(
                        o_all[:, t * 128:(t + 1) * 128], osum[:, 0:128],
                        recip[:])

                nc.sync.dma_start(
                    o_dram[b].rearrange("(c p) d -> p c d", p=128),
                    o_all[:].rearrange("p (c d) -> p c d", d=128))

    nc.compile()
    return nc


def _get_nc():
    if "nc" not in _CACHE:
        _CACHE["nc"] = _build()
    return _CACHE["nc"]


def run(queries, keys, values, trace=False, tmpdir=None):
    """Run on 8 cores; returns (output, BassKernelResults)."""
    from concourse.bass_utils import run_bass_kernel_spmd

    nc = _get_nc()
    queries = np.ascontiguousarray(queries, dtype=np.float32)
    keys = np.ascontiguousarray(keys, dtype=np.float32)
    values = np.ascontiguousarray(values, dtype=np.float32)
    in_maps = []
    for c in range(N_CORES):
        s = slice(c * B_LOC, (c + 1) * B_LOC)
        in_maps.append({
            "queries": queries[s],
            "keys": keys[s],
            "values": values[s],
        })
    res = run_bass_kernel_spmd(nc, in_maps, core_ids=list(range(N_CORES)),
                               trace=trace, tmpdir=tmpdir)
    out = np.concatenate([res.results[c]["out"] for c in range(N_CORES)], axis=0)
    return out, res


def kernel(queries, keys, values):
    out, _ = run(queries, keys, values)
    return out
